# revision 2
# baseline (speedup 1.0000x reference)
"""CrissCross(actually dense)Attention Trainium2 kernel.

Reference computation (per batch b):
    q = Wq @ x  [32, N],  k = Wk @ x  [32, N],  v = Wv @ x  [256, N],  N = 4096
    S[m, n] = softmax_n(q[:, m] . k[:, n])     (rows = queries, normalized over keys)
    out[c, n] = sum_m v[c, m] * S[m, n] + x[c, n]

Sharding: 8 cores = 4 batches x 2 query-halves. Each core handles 2048 queries m
of one batch and produces the partial out[c, n] = sum_{m in half} v[c, m] S[m, n]
over ALL n. The host sums the two partials per batch and adds the residual x.

To keep one uniform SPMD program, each core receives x with its key columns
rotated so that its own query columns are always columns [0, 2048); the partial
output comes back in rotated key order and the host un-rotates it.

Softmax is computed without max-subtraction: logits are bounded (|logit| < ~30
for these weight scales), so exp() in fp32 is safe. Row sums come for free from
the activation engine's accum_out.
"""

import numpy as np

B, C, HH, WW = 4, 256, 64, 64
N = HH * WW          # 4096 keys
CB = 32              # bottleneck channels
NCORES = 8
NL = N // 2          # 2048 local queries per core
TQ = NL // 128       # 16 query tiles of 128
GROUP = 4            # query tiles per PSUM-accumulation group

_CACHE = {}


def _build_program(big="float32r", proj="float32"):
    """Build + compile the per-core Bass program.

    big:  matmul input dtype view for the two big matmuls (logits, out)
    proj: matmul input dtype view for the q/k/v projections
    """
    import concourse.mybir as mybir
    import concourse.tile as tile
    from concourse import bacc
    from concourse.bass import ds

    f32 = mybir.dt.float32
    big_dt = getattr(mybir.dt, big)
    proj_dt = getattr(mybir.dt, proj)
    AF = mybir.ActivationFunctionType

    nc = bacc.Bacc(
        "TRN2", target_bir_lowering=False, debug=False, enable_asserts=False
    )

    x_d = nc.dram_tensor("x", [C, N], f32, kind="ExternalInput")
    wq_d = nc.dram_tensor("wq", [C, CB], f32, kind="ExternalInput")   # Wq.T
    wk_d = nc.dram_tensor("wk", [C, CB], f32, kind="ExternalInput")   # Wk.T
    wv_d = nc.dram_tensor("wv", [C, C], f32, kind="ExternalInput")    # Wv.T
    bq_d = nc.dram_tensor("bq", [CB, 1], f32, kind="ExternalInput")
    bk_d = nc.dram_tensor("bk", [CB, 1], f32, kind="ExternalInput")
    bv_d = nc.dram_tensor("bv", [128, C], f32, kind="ExternalInput")  # broadcast
    out_d = nc.dram_tensor("out", [C, N], f32, kind="ExternalOutput")

    def bc(ap, dt):
        return ap.bitcast(dt) if dt != f32 else ap

    with tile.TileContext(nc) as tc:
        with (
            tc.tile_pool(name="const", bufs=1) as cpool,
            tc.tile_pool(name="big", bufs=1) as bpool,
            tc.tile_pool(name="pp", bufs=6) as ppool,
            tc.tile_pool(name="vs", bufs=5) as vpool,
            tc.tile_pool(name="stat", bufs=6) as spool,
            tc.tile_pool(name="psl", bufs=1, space="PSUM") as psl,
            tc.tile_pool(name="pso", bufs=2, space="PSUM") as pso,
        ):
            # ---- constants ----
            wq_t = cpool.tile([128, 2, CB], f32, tag="wq")
            nc.sync.dma_start(out=wq_t, in_=wq_d.ap().rearrange("(a p) m -> p a m", p=128))
            wk_t = cpool.tile([128, 2, CB], f32, tag="wk")
            nc.sync.dma_start(out=wk_t, in_=wk_d.ap().rearrange("(a p) m -> p a m", p=128))
            wv_t = cpool.tile([128, 2, C], f32, tag="wv")
            nc.sync.dma_start(out=wv_t, in_=wv_d.ap().rearrange("(a p) m -> p a m", p=128))
            bq_t = cpool.tile([CB, 1], f32, tag="bq")
            nc.sync.dma_start(out=bq_t, in_=bq_d.ap())
            bk_t = cpool.tile([CB, 1], f32, tag="bk")
            nc.sync.dma_start(out=bk_t, in_=bk_d.ap())
            bv_t = cpool.tile([128, C], f32, tag="bv")
            nc.sync.dma_start(out=bv_t, in_=bv_d.ap())

            # ---- persistent SBUF tensors ----
            k_t = bpool.tile([CB, N], big_dt, tag="k")        # keys  [32, 4096]
            q_t = bpool.tile([CB, NL], big_dt, tag="q")       # local queries [32, 2048]
            vt_t = bpool.tile([128, TQ * C], f32, tag="vt")  # v^T local [m, c] tiles
            acc0 = bpool.tile([128, N], f32, tag="acc0")   # out rows 0..127
            acc1 = bpool.tile([128, N], f32, tag="acc1")   # out rows 128..255

            # ---- prologue: q, k, v^T projections ----
            for cc in range(4):  # 1024-column chunks of x
                x0 = ppool.tile([128, 1024], f32, tag="P", name=f"x0_{cc}")
                nc.sync.dma_start(out=x0, in_=x_d.ap()[0:128, ds(cc * 1024, 1024)])
                x1 = ppool.tile([128, 1024], f32, tag="P", name=f"x1_{cc}")
                nc.sync.dma_start(out=x1, in_=x_d.ap()[128:256, ds(cc * 1024, 1024)])
                for s in range(2):  # 512-column sub-chunks
                    col = cc * 1024 + s * 512
                    pk = pso.tile([CB, 512], f32, tag="o", name=f"pk_{col}")
                    nc.tensor.matmul(pk, bc(wk_t[:, 0, :], proj_dt),
                                     bc(x0[:, ds(s * 512, 512)], proj_dt),
                                     start=True, stop=False)
                    nc.tensor.matmul(pk, bc(wk_t[:, 1, :], proj_dt),
                                     bc(x1[:, ds(s * 512, 512)], proj_dt),
                                     start=False, stop=True)
                    nc.scalar.activation(k_t[:, ds(col, 512)], pk, AF.Identity,
                                         bias=bk_t, scale=1.0)
                    if cc < 2:
                        pq = pso.tile([CB, 512], f32, tag="o", name=f"pq_{col}")
                        nc.tensor.matmul(pq, bc(wq_t[:, 0, :], proj_dt),
                                         bc(x0[:, ds(s * 512, 512)], proj_dt),
                                         start=True, stop=False)
                        nc.tensor.matmul(pq, bc(wq_t[:, 1, :], proj_dt),
                                         bc(x1[:, ds(s * 512, 512)], proj_dt),
                                         start=False, stop=True)
                        nc.scalar.activation(q_t[:, ds(col, 512)], pq, AF.Identity,
                                             bias=bq_t, scale=1.0)
                if cc < 2:
                    for i in range(8):  # 128-col tiles -> v^T query tiles
                        t = cc * 8 + i
                        pv = pso.tile([128, C], f32, tag="o", name=f"pv_{t}")
                        nc.tensor.matmul(pv, bc(x0[:, ds(i * 128, 128)], proj_dt),
                                         bc(wv_t[:, 0, :], proj_dt),
                                         start=True, stop=False)
                        nc.tensor.matmul(pv, bc(x1[:, ds(i * 128, 128)], proj_dt),
                                         bc(wv_t[:, 1, :], proj_dt),
                                         start=False, stop=True)
                        nc.vector.tensor_add(vt_t[:, ds(t * C, C)], pv, bv_t)

            # ---- main loop: softmax rows + out accumulation ----
            p_tiles = [None] * TQ
            vs_tiles = [None] * TQ
            for g in range(TQ // GROUP):
                for tt in range(GROUP):
                    t = g * GROUP + tt
                    p_t = ppool.tile([128, N], big_dt, tag="P", name=f"p_{t}")
                    s0 = spool.tile([128, 1], f32, tag="s0", name=f"s0_{t}")
                    s1 = spool.tile([128, 1], f32, tag="s1", name=f"s1_{t}")
                    inv = spool.tile([128, 1], f32, tag="inv", name=f"inv_{t}")
                    for h2 in range(2):
                        pl = psl.tile([128, 2048], f32, tag="l", name=f"pl_{t}_{h2}")
                        for j in range(4):
                            nc.tensor.matmul(
                                pl[:, ds(j * 512, 512)],
                                q_t[:, ds(t * 128, 128)],
                                k_t[:, ds(h2 * 2048 + j * 512, 512)],
                                start=True, stop=True)
                        nc.scalar.activation(p_t[:, ds(h2 * 2048, 2048)], pl,
                                             AF.Exp, accum_out=(s0 if h2 == 0 else s1))
                    nc.vector.tensor_add(inv, s0, s1)
                    nc.vector.reciprocal(inv, inv)
                    vs_t = vpool.tile([128, C], big_dt, tag="vs", name=f"vs_{t}")
                    nc.scalar.mul(vs_t, vt_t[:, ds(t * C, C)], inv)
                    p_tiles[t] = p_t
                    vs_tiles[t] = vs_t

                for qc in range(4):      # 1024 key-columns per psum tile
                    for c2 in range(2):  # output channel tile
                        po = pso.tile([128, 1024], f32, tag="o", name=f"po_{g}_{qc}_{c2}")
                        for jj in range(2):
                            for tt in range(GROUP):
                                t = g * GROUP + tt
                                nc.tensor.matmul(
                                    po[:, ds(jj * 512, 512)],
                                    vs_tiles[t][:, ds(c2 * 128, 128)],
                                    p_tiles[t][:, ds(qc * 1024 + jj * 512, 512)],
                                    start=(tt == 0), stop=(tt == GROUP - 1))
                        acc = acc0 if c2 == 0 else acc1
                        dst = acc[:, ds(qc * 1024, 1024)]
                        if g == 0:
                            nc.vector.tensor_copy(dst, po)
                        else:
                            nc.vector.tensor_add(dst, dst, po)
                        if g == TQ // GROUP - 1:
                            nc.sync.dma_start(
                                out=out_d.ap()[c2 * 128:(c2 + 1) * 128, ds(qc * 1024, 1024)],
                                in_=dst)

    nc.compile()
    return nc


def _get_program(**kw):
    key = tuple(sorted(kw.items()))
    if key not in _CACHE:
        _CACHE[key] = _build_program(**kw)
    return _CACHE[key]


def _make_in_maps(x, Wq, bq, Wk, bk, Wv, bv):
    wq = np.ascontiguousarray(Wq.T, np.float32)
    wk = np.ascontiguousarray(Wk.T, np.float32)
    wv = np.ascontiguousarray(Wv.T, np.float32)
    bq2 = np.ascontiguousarray(bq.reshape(CB, 1), np.float32)
    bk2 = np.ascontiguousarray(bk.reshape(CB, 1), np.float32)
    bv2 = np.ascontiguousarray(np.broadcast_to(bv[None, :], (128, C)), np.float32)
    in_maps = []
    for core in range(NCORES):
        b, h = core // 2, core % 2
        xb = x[b].reshape(C, N)
        xrot = np.ascontiguousarray(np.roll(xb, -NL * h, axis=1))
        in_maps.append({"x": xrot, "wq": wq, "wk": wk, "wv": wv,
                        "bq": bq2, "bk": bk2, "bv": bv2})
    return in_maps


def _assemble(x, parts):
    y = np.empty((B, C, N), np.float32)
    for b in range(B):
        p0 = parts[2 * b]
        p1 = np.roll(parts[2 * b + 1], NL, axis=1)
        y[b] = p0 + p1 + x[b].reshape(C, N)
    return y.reshape(B, C, HH, WW)


def kernel(x, Wq, bq, Wk, bk, Wv, bv, _trace=False, _trace_kwargs=None):
    from concourse.bass_utils import run_bass_kernel_spmd

    x = np.asarray(x, np.float32)
    nc = _get_program()
    in_maps = _make_in_maps(x, np.asarray(Wq, np.float32), np.asarray(bq, np.float32),
                            np.asarray(Wk, np.float32), np.asarray(bk, np.float32),
                            np.asarray(Wv, np.float32), np.asarray(bv, np.float32))
    res = run_bass_kernel_spmd(nc, in_maps, core_ids=list(range(NCORES)),
                               trace=_trace, **(_trace_kwargs or {}))
    parts = [r["out"] for r in res.results]
    out = _assemble(x, parts)
    if _trace:
        return out, res
    return out
